# revision 18
# baseline (speedup 1.0000x reference)
"""Trainium2 Bass kernel for nn_ConditionalSelfAttention.

Reference computation (B=16, L=1024, C=512, H=8, D=64):
    qc = query @ Wqc.T + bqc ; qp = query_pos @ Wqp.T + bqp
    kc = query @ Wkc.T + bkc ; kp = query_pos @ Wkp.T + bkp
    v  = query @ Wv.T  + bv
    q = split_heads(qc+qp) * D**-0.5 ; k = split_heads(kc+kp)
    out = softmax(q @ k.T) @ split_heads(v)
    y = query + merge_heads(out) @ Wo.T + bo

Sharding: data-parallel over batch B across the 8 cores (2 batches/core),
no collectives.

Device dataflow (per core, per batch of 1024 tokens):
  - host pre-transposes query/query_pos to [C, T] and all weights to
    [c_in, c_out], and pre-adds bo into the residual; all matmul operands
    are fp32r (TF32-like single-pass PE mode).
  - q/k projections produce TRANSPOSED activations qT/kT [c_out, tok] by
    psum-accumulating Wc.T@X.T + Wp.T@P.T; biases are per-partition adds
    on the psum->sbuf evacuation.
  - v projection produces NATURAL layout [tok, c_out] (lhsT = X.T chunk),
    written head-major with a column of ones appended per head.
  - scores: attnT[k,q] = kT.T @ qT per head (contraction dim D=64; the two
    heads of a 128-channel pair ride different PE row-groups). exp via ACT
    with the 1/sqrt(D) scale folded in.
  - attn@V: outT[d,q] = [V|1].T @ exp_attnT accumulated over k-tiles; the
    ones column makes psum row 64 the softmax denominator. Normalization:
    reciprocal_approx_fast + DMA partition-broadcast + multiply-on-evac.
  - y = outT.T @ Wo.T + (query + bo), evacuated with the residual add.
"""

import numpy as np

import concourse.bass as bass
import concourse.tile as tile
from concourse import bacc, mybir
from concourse import bass_utils

B, L, C, H, D = 16, 1024, 512, 8, 64
NCORES = 8
BPC = B // NCORES  # batches per core
T = BPC * L  # tokens per core
SCALE = float(D) ** -0.5
P = 128
NCT = C // P  # c-tiles (=4); also number of head pairs
NJ = L // P  # 128-token tiles per batch (=8)
f32 = mybir.dt.float32
f32r = mybir.dt.float32r
AL = mybir.AluOpType


def build_kernel(dbg=False):
    nc = bacc.Bacc("TRN2", debug=False, num_devices=NCORES)

    xt = nc.dram_tensor("xt", [C, T], f32r, kind="ExternalInput")
    pt = nc.dram_tensor("pt", [C, T], f32r, kind="ExternalInput")
    xres = nc.dram_tensor("xres", [T, C], f32, kind="ExternalInput")
    wqct = nc.dram_tensor("wqct", [C, C], f32r, kind="ExternalInput")
    wqpt = nc.dram_tensor("wqpt", [C, C], f32r, kind="ExternalInput")
    wkct = nc.dram_tensor("wkct", [C, C], f32r, kind="ExternalInput")
    wkpt = nc.dram_tensor("wkpt", [C, C], f32r, kind="ExternalInput")
    wvt = nc.dram_tensor("wvt", [C, C], f32r, kind="ExternalInput")
    wot = nc.dram_tensor("wot", [C, C], f32r, kind="ExternalInput")
    bq = nc.dram_tensor("bq", [C], f32, kind="ExternalInput")
    bk = nc.dram_tensor("bk", [C], f32, kind="ExternalInput")
    bv = nc.dram_tensor("bv", [C], f32, kind="ExternalInput")
    y = nc.dram_tensor("y", [T, C], f32, kind="ExternalOutput")
    if dbg:
        d_qt = nc.dram_tensor("d_qt", [P, NCT, L], f32r, kind="ExternalOutput")
        d_kt = nc.dram_tensor("d_kt", [P, NCT, L], f32r, kind="ExternalOutput")
        d_vn = nc.dram_tensor("d_vn", [P, NJ, H, D + 1], f32r, kind="ExternalOutput")
        d_exp = nc.dram_tensor("d_exp", [P, L], f32r, kind="ExternalOutput")
        d_po = nc.dram_tensor("d_po", [D + 1, 512], f32, kind="ExternalOutput")
        d_rr = nc.dram_tensor("d_rr", [1, 512], f32, kind="ExternalOutput")

    with tile.TileContext(nc) as tc:
        with (
            tc.tile_pool(name="const", bufs=1) as cpool,
            tc.tile_pool(name="xp", bufs=1) as xpool,
            tc.tile_pool(name="qk", bufs=1) as qkpool,
            tc.tile_pool(name="vn", bufs=1) as vpool,
            tc.tile_pool(name="exp", bufs=6) as epool,
            tc.tile_pool(name="osb", bufs=5) as opool,
            tc.tile_pool(name="rr", bufs=2) as rpool,
            tc.tile_pool(name="io", bufs=4) as iopool,
            tc.tile_pool(name="dsc", bufs=8, space="DRAM") as dpool,
            tc.tile_pool(name="ps", bufs=2, space="PSUM") as pspool,
            tc.tile_pool(name="pssc", bufs=2, space="PSUM") as scpool,
            tc.tile_pool(name="psout", bufs=2, space="PSUM") as povpool,
        ):
            # ---- constants ----
            def load_w(t):
                w = cpool.tile([P, NCT, C], f32r, tag=f"w_{t.name}")
                nc.sync.dma_start(w[:], t.ap().rearrange("(ko p) co -> p ko co", p=P))
                return w

            w_qc, w_qp = load_w(wqct), load_w(wqpt)
            w_kc, w_kp = load_w(wkct), load_w(wkpt)
            w_v, w_o = load_w(wvt), load_w(wot)

            bq_s = cpool.tile([P, NCT], f32, tag="bq")
            bk_s = cpool.tile([P, NCT], f32, tag="bk")
            nc.sync.dma_start(bq_s[:], bq.ap().rearrange("(ct p) -> p ct", p=P))
            nc.sync.dma_start(bk_s[:], bk.ap().rearrange("(ct p) -> p ct", p=P))
            bv_b = cpool.tile([P, C], f32, tag="bvb")
            nc.sync.dma_start(bv_b[:], bv.ap()[None, :].to_broadcast((P, C)))

            for b in range(BPC):
                tok0 = b * L
                # ---- load transposed activations for this batch ----
                xt_b = xpool.tile([P, NCT, L], f32r, tag="xt")
                pt_b = xpool.tile([P, NCT, L], f32r, tag="pt")
                nc.sync.dma_start(
                    xt_b[:],
                    xt.ap()[:, tok0 : tok0 + L].rearrange("(ko p) t -> p ko t", p=P),
                )
                nc.sync.dma_start(
                    pt_b[:],
                    pt.ap()[:, tok0 : tok0 + L].rearrange("(ko p) t -> p ko t", p=P),
                )

                # ---- q/k projections (transposed outputs) ----
                qT = qkpool.tile([P, NCT, L], f32r, tag="qT")
                kT = qkpool.tile([P, NCT, L], f32r, tag="kT")
                for dst, wc, wp, bias in (
                    (qT, w_qc, w_qp, bq_s),
                    (kT, w_kc, w_kp, bk_s),
                ):
                    for ct in range(NCT):
                        for s in range(L // 512):
                            ps = pspool.tile([P, 512], f32, tag="ps")
                            for ko in range(NCT):
                                nc.tensor.matmul(
                                    ps[:],
                                    wc[:, ko, ct * P : (ct + 1) * P],
                                    xt_b[:, ko, s * 512 : (s + 1) * 512],
                                    start=(ko == 0),
                                    stop=False,
                                )
                            for ko in range(NCT):
                                nc.tensor.matmul(
                                    ps[:],
                                    wp[:, ko, ct * P : (ct + 1) * P],
                                    pt_b[:, ko, s * 512 : (s + 1) * 512],
                                    start=False,
                                    stop=(ko == NCT - 1),
                                )
                            nc.vector.tensor_scalar_add(
                                dst[:, ct, s * 512 : (s + 1) * 512],
                                ps[:],
                                bias[:, ct : ct + 1],
                            )

                # ---- v projection (natural layout, head-major, +ones col) ----
                v_nat = vpool.tile([P, NJ, H, D + 1], f32r, tag="vn")
                # ones column: in0*0 + 1 (memset on this strided region is
                # rejected by codegen)
                nc.vector.tensor_scalar(
                    v_nat[:, :, :, D : D + 1],
                    bv_b[:, 0 : NJ * H].rearrange("p (a b) -> p a b", b=H)[:, :, :, None],
                    0.0,
                    1.0,
                    AL.mult,
                    AL.add,
                )
                for tt in range(NJ):
                    ps = pspool.tile([P, 512], f32, tag="ps")
                    for ko in range(NCT):
                        nc.tensor.matmul(
                            ps[:],
                            xt_b[:, ko, tt * P : (tt + 1) * P],
                            w_v[:, ko, :],
                            start=(ko == 0),
                            stop=(ko == NCT - 1),
                        )
                    nc.vector.tensor_tensor(
                        v_nat[:, tt, :, 0:D],
                        ps[:].rearrange("p (h d) -> p h d", d=D),
                        bv_b[:].rearrange("p (h d) -> p h d", d=D),
                        AL.add,
                    )

                if dbg and b == 0:
                    nc.sync.dma_start(d_qt.ap(), qT[:])
                    nc.sync.dma_start(d_kt.ap(), kT[:])
                    nc.sync.dma_start(d_vn.ap(), v_nat[:])

                # ---- attention per head-pair, heads sequential ----
                out_sb = {}
                for hp in range(NCT):
                    osb = opool.tile([P, L], f32r, tag="osb")
                    out_sb[hp] = osb
                    for h01 in range(2):
                        h = hp * 2 + h01
                        prow = slice(h01 * D, (h01 + 1) * D)
                        # scores + exp per k-tile
                        exps = []
                        for j in range(NJ):
                            psc = scpool.tile([P, L], f32, tag="sc")
                            for s in range(L // 512):
                                nc.tensor.matmul(
                                    psc[:, s * 512 : (s + 1) * 512],
                                    kT[prow, hp, j * P : (j + 1) * P],
                                    qT[prow, hp, s * 512 : (s + 1) * 512],
                                    start=True,
                                    stop=True,
                                )
                            et = epool.tile([P, L], f32r, tag="exp")
                            nc.scalar.activation(
                                et[:],
                                psc[:],
                                mybir.ActivationFunctionType.Exp,
                                scale=SCALE,
                            )
                            exps.append(et)
                            if dbg and b == 0 and hp == 0 and h01 == 0 and j == 0:
                                nc.sync.dma_start(d_exp.ap(), et[:])

                        # attn @ [V|1]: accumulate over k-tiles; per-j order
                        # frees each exp slot after its two matmuls
                        pos = []
                        for s in range(L // 512):
                            po_s = povpool.tile([D + 1, 512], f32, tag="po", name=f"po_{s}")
                            pos.append(po_s)
                        for j in range(NJ):
                            for s in range(L // 512):
                                nc.tensor.matmul(
                                    pos[s][:],
                                    v_nat[:, j, h, :],
                                    exps[j][:, s * 512 : (s + 1) * 512],
                                    start=(j == 0),
                                    stop=(j == NJ - 1),
                                )
                        for s in range(L // 512):
                            po = pos[s]
                            if dbg and b == 0 and hp == 0 and h01 == 0 and s == 0:
                                dbg_sb = rpool.tile([D + 1, 512], f32, tag="rb")
                                nc.vector.tensor_copy(dbg_sb[:], po[:])
                                nc.sync.dma_start(d_po.ap(), dbg_sb[:])
                            # custom DVE ops only work at partition base 0:
                            # shift-copy the rowsum row down first
                            rraw = rpool.tile([1, 512], f32, tag="rraw")
                            nc.vector.tensor_copy(rraw[0:1, :], po[D : D + 1, :])
                            rrec = rpool.tile([1, 512], f32, tag="rrec")
                            nc.vector.reciprocal_approx_fast(rrec[:], rraw[:])
                            if dbg and b == 0 and hp == 0 and h01 == 0 and s == 0:
                                nc.sync.dma_start(d_rr.ap(), rrec[:])
                            dscr = dpool.tile([1, 512], f32, tag="dscr")
                            nc.sync.dma_start(dscr[:], rrec[:])
                            rb = rpool.tile([D, 512], f32, tag="rb")
                            nc.sync.dma_start(
                                rb[:], dscr[0:1, :].to_broadcast((D, 512))
                            )
                            nc.vector.tensor_tensor(
                                osb[prow, s * 512 : (s + 1) * 512],
                                po[0:D, :],
                                rb[:],
                                AL.mult,
                            )

                # ---- output projection + residual ----
                for tt in range(NJ):
                    psy = pspool.tile([P, 512], f32, tag="ps")
                    for hp in range(NCT):
                        nc.tensor.matmul(
                            psy[:],
                            out_sb[hp][:, tt * P : (tt + 1) * P],
                            w_o[:, hp, :],
                            start=(hp == 0),
                            stop=(hp == NCT - 1),
                        )
                    xr = iopool.tile([P, C], f32, tag="xr")
                    nc.sync.dma_start(
                        xr[:], xres.ap()[tok0 + tt * P : tok0 + (tt + 1) * P, :]
                    )
                    ysb = iopool.tile([P, C], f32, tag="ysb")
                    nc.vector.tensor_tensor(ysb[:], psy[:], xr[:], AL.add)
                    nc.sync.dma_start(
                        y.ap()[tok0 + tt * P : tok0 + (tt + 1) * P, :], ysb[:]
                    )

    nc.compile()
    return nc


_NC_CACHE = None


def _get_nc():
    global _NC_CACHE
    if _NC_CACHE is None:
        _NC_CACHE = build_kernel()
    return _NC_CACHE


def make_in_maps(query, query_pos, Wqc, bqc, Wqp, bqp, Wkc, bkc, Wkp, bkp, Wv, bv, Wo, bo):
    """Host-side sharding + layout prep: one input map per core."""
    query = np.asarray(query, dtype=np.float32)
    query_pos = np.asarray(query_pos, dtype=np.float32)
    shared = {
        "wqct": np.ascontiguousarray(np.asarray(Wqc, np.float32).T),
        "wqpt": np.ascontiguousarray(np.asarray(Wqp, np.float32).T),
        "wkct": np.ascontiguousarray(np.asarray(Wkc, np.float32).T),
        "wkpt": np.ascontiguousarray(np.asarray(Wkp, np.float32).T),
        "wvt": np.ascontiguousarray(np.asarray(Wv, np.float32).T),
        "wot": np.ascontiguousarray(np.asarray(Wo, np.float32).T),
        "bq": np.asarray(bqc, np.float32) + np.asarray(bqp, np.float32),
        "bk": np.asarray(bkc, np.float32) + np.asarray(bkp, np.float32),
        "bv": np.asarray(bv, np.float32),
    }
    in_maps = []
    for c in range(NCORES):
        xc = query[c * BPC : (c + 1) * BPC].reshape(T, C)
        pc = query_pos[c * BPC : (c + 1) * BPC].reshape(T, C)
        in_maps.append(
            dict(
                shared,
                xt=np.ascontiguousarray(xc.T),
                pt=np.ascontiguousarray(pc.T),
                xres=xc + np.asarray(bo, np.float32)[None, :],
            )
        )
    return in_maps


def kernel(**inputs) -> np.ndarray:
    nc = _get_nc()
    in_maps = make_in_maps(**inputs)
    res = bass_utils.run_bass_kernel_spmd(nc, in_maps, core_ids=list(range(NCORES)))
    out = np.concatenate([r["y"].reshape(BPC, L, C) for r in res.results], axis=0)
    return out


# revision 20
# speedup vs baseline: 1.0350x; 1.0350x over previous
"""Trainium2 Bass kernel for nn_ConditionalSelfAttention.

Reference computation (B=16, L=1024, C=512, H=8, D=64):
    qc = query @ Wqc.T + bqc ; qp = query_pos @ Wqp.T + bqp
    kc = query @ Wkc.T + bkc ; kp = query_pos @ Wkp.T + bkp
    v  = query @ Wv.T  + bv
    q = split_heads(qc+qp) * D**-0.5 ; k = split_heads(kc+kp)
    out = softmax(q @ k.T) @ split_heads(v)
    y = query + merge_heads(out) @ Wo.T + bo

Sharding: data-parallel over batch B across the 8 cores (2 batches/core),
no collectives.

Device dataflow (per core, per batch of 1024 tokens):
  - host pre-transposes query/query_pos to [C, T] and all weights to
    [c_in, c_out], and pre-adds bo into the residual; all matmul operands
    are fp32r (TF32-like single-pass PE mode).
  - q/k projections produce TRANSPOSED activations qT/kT [c_out, tok] by
    psum-accumulating Wc.T@X.T + Wp.T@P.T; biases are per-partition adds
    on the psum->sbuf evacuation.
  - v projection produces NATURAL layout [tok, c_out] (lhsT = X.T chunk),
    written head-major with a column of ones appended per head.
  - scores: attnT[k,q] = kT.T @ qT per head (contraction dim D=64; the two
    heads of a 128-channel pair ride different PE row-groups). exp via ACT
    with the 1/sqrt(D) scale folded in.
  - attn@V: outT[d,q] = [V|1].T @ exp_attnT accumulated over k-tiles; the
    ones column makes psum row 64 the softmax denominator. Normalization:
    reciprocal_approx_fast + DMA partition-broadcast + multiply-on-evac.
  - y = outT.T @ Wo.T + (query + bo), evacuated with the residual add.
"""

import ml_dtypes
import numpy as np

import concourse.bass as bass
import concourse.tile as tile
from concourse import bacc, mybir
from concourse import bass_utils

B, L, C, H, D = 16, 1024, 512, 8, 64
NCORES = 8
BPC = B // NCORES  # batches per core
T = BPC * L  # tokens per core
SCALE = float(D) ** -0.5
P = 128
NCT = C // P  # c-tiles (=4); also number of head pairs
NJ = L // P  # 128-token tiles per batch (=8)
f32 = mybir.dt.float32
f32r = mybir.dt.float32r
bf16 = mybir.dt.bfloat16
AL = mybir.AluOpType


def build_kernel(dbg=False):
    nc = bacc.Bacc("TRN2", debug=False, num_devices=NCORES)

    xt = nc.dram_tensor("xt", [C, T], bf16, kind="ExternalInput")
    pt = nc.dram_tensor("pt", [C, T], bf16, kind="ExternalInput")
    xres = nc.dram_tensor("xres", [T, C], f32, kind="ExternalInput")
    wqct = nc.dram_tensor("wqct", [C, C], bf16, kind="ExternalInput")
    wqpt = nc.dram_tensor("wqpt", [C, C], bf16, kind="ExternalInput")
    wkct = nc.dram_tensor("wkct", [C, C], bf16, kind="ExternalInput")
    wkpt = nc.dram_tensor("wkpt", [C, C], bf16, kind="ExternalInput")
    wvt = nc.dram_tensor("wvt", [C, C], bf16, kind="ExternalInput")
    wot = nc.dram_tensor("wot", [C, C], bf16, kind="ExternalInput")
    bq = nc.dram_tensor("bq", [C], f32, kind="ExternalInput")
    bk = nc.dram_tensor("bk", [C], f32, kind="ExternalInput")
    bv = nc.dram_tensor("bv", [C], f32, kind="ExternalInput")
    y = nc.dram_tensor("y", [T, C], f32, kind="ExternalOutput")
    if dbg:
        d_qt = nc.dram_tensor("d_qt", [P, NCT, L], bf16, kind="ExternalOutput")
        d_kt = nc.dram_tensor("d_kt", [P, NCT, L], bf16, kind="ExternalOutput")
        d_vn = nc.dram_tensor("d_vn", [P, NJ, H, D + 1], bf16, kind="ExternalOutput")
        d_exp = nc.dram_tensor("d_exp", [P, L], bf16, kind="ExternalOutput")
        d_po = nc.dram_tensor("d_po", [D + 1, 512], f32, kind="ExternalOutput")
        d_rr = nc.dram_tensor("d_rr", [1, 512], f32, kind="ExternalOutput")

    with tile.TileContext(nc) as tc:
        with (
            tc.tile_pool(name="const", bufs=1) as cpool,
            tc.tile_pool(name="xp", bufs=1) as xpool,
            tc.tile_pool(name="qk", bufs=1) as qkpool,
            tc.tile_pool(name="vn", bufs=1) as vpool,
            tc.tile_pool(name="exp", bufs=6) as epool,
            tc.tile_pool(name="osb", bufs=5) as opool,
            tc.tile_pool(name="rr", bufs=2) as rpool,
            tc.tile_pool(name="io", bufs=4) as iopool,
            tc.tile_pool(name="dsc", bufs=8, space="DRAM") as dpool,
            tc.tile_pool(name="ps", bufs=2, space="PSUM") as pspool,
            tc.tile_pool(name="pssc", bufs=2, space="PSUM") as scpool,
            tc.tile_pool(name="psout", bufs=2, space="PSUM") as povpool,
        ):
            # ---- constants ----
            def load_w(t):
                w = cpool.tile([P, NCT, C], bf16, tag=f"w_{t.name}")
                nc.sync.dma_start(w[:], t.ap().rearrange("(ko p) co -> p ko co", p=P))
                return w

            w_qc, w_qp = load_w(wqct), load_w(wqpt)
            w_kc, w_kp = load_w(wkct), load_w(wkpt)
            w_v, w_o = load_w(wvt), load_w(wot)

            bq_s = cpool.tile([P, NCT], f32, tag="bq")
            bk_s = cpool.tile([P, NCT], f32, tag="bk")
            nc.sync.dma_start(bq_s[:], bq.ap().rearrange("(ct p) -> p ct", p=P))
            nc.sync.dma_start(bk_s[:], bk.ap().rearrange("(ct p) -> p ct", p=P))
            bv_b = cpool.tile([P, C], f32, tag="bvb")
            nc.sync.dma_start(bv_b[:], bv.ap()[None, :].to_broadcast((P, C)))

            for b in range(BPC):
                tok0 = b * L
                # ---- load transposed activations for this batch ----
                xt_b = xpool.tile([P, NCT, L], bf16, tag="xt")
                pt_b = xpool.tile([P, NCT, L], bf16, tag="pt")
                nc.sync.dma_start(
                    xt_b[:],
                    xt.ap()[:, tok0 : tok0 + L].rearrange("(ko p) t -> p ko t", p=P),
                )
                nc.sync.dma_start(
                    pt_b[:],
                    pt.ap()[:, tok0 : tok0 + L].rearrange("(ko p) t -> p ko t", p=P),
                )

                # ---- q/k projections (transposed outputs) ----
                qT = qkpool.tile([P, NCT, L], bf16, tag="qT")
                kT = qkpool.tile([P, NCT, L], bf16, tag="kT")
                for dst, wc, wp, bias in (
                    (qT, w_qc, w_qp, bq_s),
                    (kT, w_kc, w_kp, bk_s),
                ):
                    for ct in range(NCT):
                        for s in range(L // 512):
                            ps = pspool.tile([P, 512], f32, tag="ps")
                            for ko in range(NCT):
                                nc.tensor.matmul(
                                    ps[:],
                                    wc[:, ko, ct * P : (ct + 1) * P],
                                    xt_b[:, ko, s * 512 : (s + 1) * 512],
                                    start=(ko == 0),
                                    stop=False,
                                )
                            for ko in range(NCT):
                                nc.tensor.matmul(
                                    ps[:],
                                    wp[:, ko, ct * P : (ct + 1) * P],
                                    pt_b[:, ko, s * 512 : (s + 1) * 512],
                                    start=False,
                                    stop=(ko == NCT - 1),
                                )
                            nc.vector.tensor_scalar_add(
                                dst[:, ct, s * 512 : (s + 1) * 512],
                                ps[:],
                                bias[:, ct : ct + 1],
                            )

                # ---- v projection (natural layout, head-major, +ones col) ----
                v_nat = vpool.tile([P, NJ, H, D + 1], bf16, tag="vn")
                # ones column: in0*0 + 1 (memset on this strided region is
                # rejected by codegen)
                nc.vector.tensor_scalar(
                    v_nat[:, :, :, D : D + 1],
                    bv_b[:, 0 : NJ * H].rearrange("p (a b) -> p a b", b=H)[:, :, :, None],
                    0.0,
                    1.0,
                    AL.mult,
                    AL.add,
                )
                for tt in range(NJ):
                    ps = pspool.tile([P, 512], f32, tag="ps")
                    for ko in range(NCT):
                        nc.tensor.matmul(
                            ps[:],
                            xt_b[:, ko, tt * P : (tt + 1) * P],
                            w_v[:, ko, :],
                            start=(ko == 0),
                            stop=(ko == NCT - 1),
                        )
                    nc.vector.tensor_tensor(
                        v_nat[:, tt, :, 0:D],
                        ps[:].rearrange("p (h d) -> p h d", d=D),
                        bv_b[:].rearrange("p (h d) -> p h d", d=D),
                        AL.add,
                    )

                if dbg and b == 0:
                    nc.sync.dma_start(d_qt.ap(), qT[:])
                    nc.sync.dma_start(d_kt.ap(), kT[:])
                    nc.sync.dma_start(d_vn.ap(), v_nat[:])

                # ---- attention per head-pair, heads sequential ----
                out_sb = {}
                for hp in range(NCT):
                    osb = opool.tile([P, L], bf16, tag="osb")
                    out_sb[hp] = osb
                    for h01 in range(2):
                        h = hp * 2 + h01
                        prow = slice(h01 * D, (h01 + 1) * D)
                        # scores + exp per k-tile
                        exps = []
                        for j in range(NJ):
                            psc = scpool.tile([P, L], f32, tag="sc")
                            for s in range(L // 512):
                                nc.tensor.matmul(
                                    psc[:, s * 512 : (s + 1) * 512],
                                    kT[prow, hp, j * P : (j + 1) * P],
                                    qT[prow, hp, s * 512 : (s + 1) * 512],
                                    start=True,
                                    stop=True,
                                )
                            et = epool.tile([P, L], bf16, tag="exp")
                            nc.scalar.activation(
                                et[:],
                                psc[:],
                                mybir.ActivationFunctionType.Exp,
                                scale=SCALE,
                            )
                            exps.append(et)
                            if dbg and b == 0 and hp == 0 and h01 == 0 and j == 0:
                                nc.sync.dma_start(d_exp.ap(), et[:])

                        # attn @ [V|1]: accumulate over k-tiles; per-j order
                        # frees each exp slot after its two matmuls
                        pos = []
                        for s in range(L // 512):
                            po_s = povpool.tile([D + 1, 512], f32, tag="po", name=f"po_{s}")
                            pos.append(po_s)
                        for j in range(NJ):
                            for s in range(L // 512):
                                nc.tensor.matmul(
                                    pos[s][:],
                                    v_nat[:, j, h, :],
                                    exps[j][:, s * 512 : (s + 1) * 512],
                                    start=(j == 0),
                                    stop=(j == NJ - 1),
                                )
                        for s in range(L // 512):
                            po = pos[s]
                            if dbg and b == 0 and hp == 0 and h01 == 0 and s == 0:
                                dbg_sb = rpool.tile([D + 1, 512], f32, tag="rb")
                                nc.vector.tensor_copy(dbg_sb[:], po[:])
                                nc.sync.dma_start(d_po.ap(), dbg_sb[:])
                            # custom DVE ops only work at partition base 0:
                            # shift-copy the rowsum row down first
                            rraw = rpool.tile([1, 512], f32, tag="rraw")
                            nc.vector.tensor_copy(rraw[0:1, :], po[D : D + 1, :])
                            rrec = rpool.tile([1, 512], f32, tag="rrec")
                            nc.vector.reciprocal_approx_fast(rrec[:], rraw[:])
                            if dbg and b == 0 and hp == 0 and h01 == 0 and s == 0:
                                nc.sync.dma_start(d_rr.ap(), rrec[:])
                            dscr = dpool.tile([1, 512], f32, tag="dscr")
                            nc.sync.dma_start(dscr[:], rrec[:])
                            rb = rpool.tile([D, 512], f32, tag="rb")
                            nc.sync.dma_start(
                                rb[:], dscr[0:1, :].to_broadcast((D, 512))
                            )
                            nc.vector.tensor_tensor(
                                osb[prow, s * 512 : (s + 1) * 512],
                                po[0:D, :],
                                rb[:],
                                AL.mult,
                            )

                # ---- output projection + residual ----
                for tt in range(NJ):
                    psy = pspool.tile([P, 512], f32, tag="ps")
                    for hp in range(NCT):
                        nc.tensor.matmul(
                            psy[:],
                            out_sb[hp][:, tt * P : (tt + 1) * P],
                            w_o[:, hp, :],
                            start=(hp == 0),
                            stop=(hp == NCT - 1),
                        )
                    xr = iopool.tile([P, C], f32, tag="xr")
                    nc.sync.dma_start(
                        xr[:], xres.ap()[tok0 + tt * P : tok0 + (tt + 1) * P, :]
                    )
                    ysb = iopool.tile([P, C], f32, tag="ysb")
                    nc.vector.tensor_tensor(ysb[:], psy[:], xr[:], AL.add)
                    nc.sync.dma_start(
                        y.ap()[tok0 + tt * P : tok0 + (tt + 1) * P, :], ysb[:]
                    )

    nc.compile()
    return nc


_NC_CACHE = None


def _get_nc():
    global _NC_CACHE
    if _NC_CACHE is None:
        _NC_CACHE = build_kernel()
    return _NC_CACHE


def make_in_maps(query, query_pos, Wqc, bqc, Wqp, bqp, Wkc, bkc, Wkp, bkp, Wv, bv, Wo, bo):
    """Host-side sharding + layout prep: one input map per core."""
    query = np.asarray(query, dtype=np.float32)
    query_pos = np.asarray(query_pos, dtype=np.float32)
    shared = {
        "wqct": np.ascontiguousarray(np.asarray(Wqc, np.float32).T.astype(ml_dtypes.bfloat16)),
        "wqpt": np.ascontiguousarray(np.asarray(Wqp, np.float32).T.astype(ml_dtypes.bfloat16)),
        "wkct": np.ascontiguousarray(np.asarray(Wkc, np.float32).T.astype(ml_dtypes.bfloat16)),
        "wkpt": np.ascontiguousarray(np.asarray(Wkp, np.float32).T.astype(ml_dtypes.bfloat16)),
        "wvt": np.ascontiguousarray(np.asarray(Wv, np.float32).T.astype(ml_dtypes.bfloat16)),
        "wot": np.ascontiguousarray(np.asarray(Wo, np.float32).T.astype(ml_dtypes.bfloat16)),
        "bq": np.asarray(bqc, np.float32) + np.asarray(bqp, np.float32),
        "bk": np.asarray(bkc, np.float32) + np.asarray(bkp, np.float32),
        "bv": np.asarray(bv, np.float32),
    }
    in_maps = []
    for c in range(NCORES):
        xc = query[c * BPC : (c + 1) * BPC].reshape(T, C)
        pc = query_pos[c * BPC : (c + 1) * BPC].reshape(T, C)
        in_maps.append(
            dict(
                shared,
                xt=np.ascontiguousarray(xc.T.astype(ml_dtypes.bfloat16)),
                pt=np.ascontiguousarray(pc.T.astype(ml_dtypes.bfloat16)),
                xres=xc + np.asarray(bo, np.float32)[None, :],
            )
        )
    return in_maps


def kernel(**inputs) -> np.ndarray:
    nc = _get_nc()
    in_maps = make_in_maps(**inputs)
    res = bass_utils.run_bass_kernel_spmd(nc, in_maps, core_ids=list(range(NCORES)))
    out = np.concatenate([r["y"].reshape(BPC, L, C) for r in res.results], axis=0)
    return out


# revision 21
# speedup vs baseline: 1.0859x; 1.0492x over previous
"""Trainium2 Bass kernel for nn_ConditionalSelfAttention.

Reference computation (B=16, L=1024, C=512, H=8, D=64):
    qc = query @ Wqc.T + bqc ; qp = query_pos @ Wqp.T + bqp
    kc = query @ Wkc.T + bkc ; kp = query_pos @ Wkp.T + bkp
    v  = query @ Wv.T  + bv
    q = split_heads(qc+qp) * D**-0.5 ; k = split_heads(kc+kp)
    out = softmax(q @ k.T) @ split_heads(v)
    y = query + merge_heads(out) @ Wo.T + bo

Sharding: data-parallel over batch B across the 8 cores (2 batches/core),
no collectives.

Device dataflow (per core, per batch of 1024 tokens):
  - host pre-transposes query/query_pos to [C, T] and all weights to
    [c_in, c_out], and pre-adds bo into the residual; all matmul operands
    are fp32r (TF32-like single-pass PE mode).
  - q/k projections produce TRANSPOSED activations qT/kT [c_out, tok] by
    psum-accumulating Wc.T@X.T + Wp.T@P.T; biases are per-partition adds
    on the psum->sbuf evacuation.
  - v projection produces NATURAL layout [tok, c_out] (lhsT = X.T chunk),
    written head-major with a column of ones appended per head.
  - scores: attnT[k,q] = kT.T @ qT per head (contraction dim D=64; the two
    heads of a 128-channel pair ride different PE row-groups). exp via ACT
    with the 1/sqrt(D) scale folded in.
  - attn@V: outT[d,q] = [V|1].T @ exp_attnT accumulated over k-tiles; the
    ones column makes psum row 64 the softmax denominator. Normalization:
    reciprocal_approx_fast + DMA partition-broadcast + multiply-on-evac.
  - y = outT.T @ Wo.T + (query + bo), evacuated with the residual add.
"""

import ml_dtypes
import numpy as np

import concourse.bass as bass
import concourse.tile as tile
from concourse import bacc, mybir
from concourse import bass_utils

B, L, C, H, D = 16, 1024, 512, 8, 64
NCORES = 8
BPC = B // NCORES  # batches per core
T = BPC * L  # tokens per core
SCALE = float(D) ** -0.5
P = 128
NCT = C // P  # c-tiles (=4); also number of head pairs
NJ = L // P  # 128-token tiles per batch (=8)
f32 = mybir.dt.float32
f32r = mybir.dt.float32r
bf16 = mybir.dt.bfloat16
AL = mybir.AluOpType


def build_kernel(dbg=False):
    nc = bacc.Bacc("TRN2", debug=False, num_devices=NCORES)

    xt = nc.dram_tensor("xt", [C, T], bf16, kind="ExternalInput")
    pt = nc.dram_tensor("pt", [C, T], bf16, kind="ExternalInput")
    xres = nc.dram_tensor("xres", [T, C], f32, kind="ExternalInput")
    wqct = nc.dram_tensor("wqct", [C, C], bf16, kind="ExternalInput")
    wqpt = nc.dram_tensor("wqpt", [C, C], bf16, kind="ExternalInput")
    wkct = nc.dram_tensor("wkct", [C, C], bf16, kind="ExternalInput")
    wkpt = nc.dram_tensor("wkpt", [C, C], bf16, kind="ExternalInput")
    wvt = nc.dram_tensor("wvt", [C, C], bf16, kind="ExternalInput")
    wot = nc.dram_tensor("wot", [C, C], bf16, kind="ExternalInput")
    bq = nc.dram_tensor("bq", [C], f32, kind="ExternalInput")
    bk = nc.dram_tensor("bk", [C], f32, kind="ExternalInput")
    bv = nc.dram_tensor("bv", [C], f32, kind="ExternalInput")
    y = nc.dram_tensor("y", [T, C], f32, kind="ExternalOutput")
    if dbg:
        d_qt = nc.dram_tensor("d_qt", [P, NCT, L], bf16, kind="ExternalOutput")
        d_kt = nc.dram_tensor("d_kt", [P, NCT, L], bf16, kind="ExternalOutput")
        d_vn = nc.dram_tensor("d_vn", [P, NJ, H, D + 1], bf16, kind="ExternalOutput")
        d_exp = nc.dram_tensor("d_exp", [P, L], bf16, kind="ExternalOutput")
        d_po = nc.dram_tensor("d_po", [D + 1, 512], f32, kind="ExternalOutput")
        d_rr = nc.dram_tensor("d_rr", [1, 512], f32, kind="ExternalOutput")

    with tile.TileContext(nc) as tc:
        with (
            tc.tile_pool(name="const", bufs=1) as cpool,
            tc.tile_pool(name="xp", bufs=2) as xpool,
            tc.tile_pool(name="qk", bufs=2) as qkpool,
            tc.tile_pool(name="vn", bufs=1) as vpool,
            tc.tile_pool(name="exp", bufs=12) as epool,
            tc.tile_pool(name="osb", bufs=5) as opool,
            tc.tile_pool(name="rr", bufs=4) as rpool,
            tc.tile_pool(name="io", bufs=6) as iopool,
            tc.tile_pool(name="dsc", bufs=8, space="DRAM") as dpool,
            tc.tile_pool(name="ps", bufs=2, space="PSUM") as pspool,
            tc.tile_pool(name="pssc", bufs=2, space="PSUM") as scpool,
            tc.tile_pool(name="psout", bufs=2, space="PSUM") as povpool,
        ):
            # ---- constants ----
            def load_w(t):
                w = cpool.tile([P, NCT, C], bf16, tag=f"w_{t.name}")
                nc.sync.dma_start(w[:], t.ap().rearrange("(ko p) co -> p ko co", p=P))
                return w

            w_qc, w_qp = load_w(wqct), load_w(wqpt)
            w_kc, w_kp = load_w(wkct), load_w(wkpt)
            w_v, w_o = load_w(wvt), load_w(wot)

            bq_s = cpool.tile([P, NCT], f32, tag="bq")
            bk_s = cpool.tile([P, NCT], f32, tag="bk")
            nc.sync.dma_start(bq_s[:], bq.ap().rearrange("(ct p) -> p ct", p=P))
            nc.sync.dma_start(bk_s[:], bk.ap().rearrange("(ct p) -> p ct", p=P))
            bv_b = cpool.tile([P, C], f32, tag="bvb")
            nc.sync.dma_start(bv_b[:], bv.ap()[None, :].to_broadcast((P, C)))

            for b in range(BPC):
                tok0 = b * L
                # ---- load transposed activations for this batch ----
                xt_b = xpool.tile([P, NCT, L], bf16, tag="xt")
                pt_b = xpool.tile([P, NCT, L], bf16, tag="pt")
                nc.sync.dma_start(
                    xt_b[:],
                    xt.ap()[:, tok0 : tok0 + L].rearrange("(ko p) t -> p ko t", p=P),
                )
                nc.sync.dma_start(
                    pt_b[:],
                    pt.ap()[:, tok0 : tok0 + L].rearrange("(ko p) t -> p ko t", p=P),
                )

                # ---- q/k projections (transposed outputs) ----
                qT = qkpool.tile([P, NCT, L], bf16, tag="qT")
                kT = qkpool.tile([P, NCT, L], bf16, tag="kT")
                for dst, wc, wp, bias in (
                    (qT, w_qc, w_qp, bq_s),
                    (kT, w_kc, w_kp, bk_s),
                ):
                    for ct in range(NCT):
                        for s in range(L // 512):
                            ps = pspool.tile([P, 512], f32, tag="ps")
                            for ko in range(NCT):
                                nc.tensor.matmul(
                                    ps[:],
                                    wc[:, ko, ct * P : (ct + 1) * P],
                                    xt_b[:, ko, s * 512 : (s + 1) * 512],
                                    start=(ko == 0),
                                    stop=False,
                                )
                            for ko in range(NCT):
                                nc.tensor.matmul(
                                    ps[:],
                                    wp[:, ko, ct * P : (ct + 1) * P],
                                    pt_b[:, ko, s * 512 : (s + 1) * 512],
                                    start=False,
                                    stop=(ko == NCT - 1),
                                )
                            nc.vector.tensor_scalar_add(
                                dst[:, ct, s * 512 : (s + 1) * 512],
                                ps[:],
                                bias[:, ct : ct + 1],
                            )

                # ---- v projection (natural layout, head-major, +ones col) ----
                v_nat = vpool.tile([P, NJ, H, D + 1], bf16, tag="vn")
                # ones column: in0*0 + 1 (memset on this strided region is
                # rejected by codegen)
                nc.vector.tensor_scalar(
                    v_nat[:, :, :, D : D + 1],
                    bv_b[:, 0 : NJ * H].rearrange("p (a b) -> p a b", b=H)[:, :, :, None],
                    0.0,
                    1.0,
                    AL.mult,
                    AL.add,
                )
                for tt in range(NJ):
                    ps = pspool.tile([P, 512], f32, tag="ps")
                    for ko in range(NCT):
                        nc.tensor.matmul(
                            ps[:],
                            xt_b[:, ko, tt * P : (tt + 1) * P],
                            w_v[:, ko, :],
                            start=(ko == 0),
                            stop=(ko == NCT - 1),
                        )
                    nc.vector.tensor_tensor(
                        v_nat[:, tt, :, 0:D],
                        ps[:].rearrange("p (h d) -> p h d", d=D),
                        bv_b[:].rearrange("p (h d) -> p h d", d=D),
                        AL.add,
                    )

                if dbg and b == 0:
                    nc.sync.dma_start(d_qt.ap(), qT[:])
                    nc.sync.dma_start(d_kt.ap(), kT[:])
                    nc.sync.dma_start(d_vn.ap(), v_nat[:])

                # ---- attention per head-pair, heads sequential ----
                out_sb = {}
                for hp in range(NCT):
                    osb = opool.tile([P, L], bf16, tag="osb")
                    out_sb[hp] = osb
                    for h01 in range(2):
                        h = hp * 2 + h01
                        prow = slice(h01 * D, (h01 + 1) * D)
                        # scores + exp per k-tile
                        exps = []
                        for j in range(NJ):
                            psc = scpool.tile([P, L], f32, tag="sc")
                            for s in range(L // 512):
                                nc.tensor.matmul(
                                    psc[:, s * 512 : (s + 1) * 512],
                                    kT[prow, hp, j * P : (j + 1) * P],
                                    qT[prow, hp, s * 512 : (s + 1) * 512],
                                    start=True,
                                    stop=True,
                                )
                            et = epool.tile([P, L], bf16, tag="exp")
                            nc.scalar.activation(
                                et[:],
                                psc[:],
                                mybir.ActivationFunctionType.Exp,
                                scale=SCALE,
                            )
                            exps.append(et)
                            if dbg and b == 0 and hp == 0 and h01 == 0 and j == 0:
                                nc.sync.dma_start(d_exp.ap(), et[:])

                        # attn @ [V|1]: accumulate over k-tiles; per-j order
                        # frees each exp slot after its two matmuls
                        pos = []
                        for s in range(L // 512):
                            po_s = povpool.tile([D + 1, 512], f32, tag="po", name=f"po_{s}")
                            pos.append(po_s)
                        for j in range(NJ):
                            for s in range(L // 512):
                                nc.tensor.matmul(
                                    pos[s][:],
                                    v_nat[:, j, h, :],
                                    exps[j][:, s * 512 : (s + 1) * 512],
                                    start=(j == 0),
                                    stop=(j == NJ - 1),
                                )
                        for s in range(L // 512):
                            po = pos[s]
                            if dbg and b == 0 and hp == 0 and h01 == 0 and s == 0:
                                dbg_sb = rpool.tile([D + 1, 512], f32, tag="rb")
                                nc.vector.tensor_copy(dbg_sb[:], po[:])
                                nc.sync.dma_start(d_po.ap(), dbg_sb[:])
                            # custom DVE ops only work at partition base 0:
                            # shift-copy the rowsum row down first
                            rraw = rpool.tile([1, 512], f32, tag="rraw")
                            nc.vector.tensor_copy(rraw[0:1, :], po[D : D + 1, :])
                            rrec = rpool.tile([1, 512], f32, tag="rrec")
                            nc.vector.reciprocal_approx_fast(rrec[:], rraw[:])
                            if dbg and b == 0 and hp == 0 and h01 == 0 and s == 0:
                                nc.sync.dma_start(d_rr.ap(), rrec[:])
                            dscr = dpool.tile([1, 512], f32, tag="dscr")
                            nc.sync.dma_start(dscr[:], rrec[:])
                            rb = rpool.tile([D, 512], f32, tag="rb")
                            nc.sync.dma_start(
                                rb[:], dscr[0:1, :].to_broadcast((D, 512))
                            )
                            nc.vector.tensor_tensor(
                                osb[prow, s * 512 : (s + 1) * 512],
                                po[0:D, :],
                                rb[:],
                                AL.mult,
                            )

                # ---- output projection + residual ----
                for tt in range(NJ):
                    psy = pspool.tile([P, 512], f32, tag="ps")
                    for hp in range(NCT):
                        nc.tensor.matmul(
                            psy[:],
                            out_sb[hp][:, tt * P : (tt + 1) * P],
                            w_o[:, hp, :],
                            start=(hp == 0),
                            stop=(hp == NCT - 1),
                        )
                    xr = iopool.tile([P, C], f32, tag="xr")
                    nc.sync.dma_start(
                        xr[:], xres.ap()[tok0 + tt * P : tok0 + (tt + 1) * P, :]
                    )
                    ysb = iopool.tile([P, C], f32, tag="ysb")
                    nc.vector.tensor_tensor(ysb[:], psy[:], xr[:], AL.add)
                    nc.sync.dma_start(
                        y.ap()[tok0 + tt * P : tok0 + (tt + 1) * P, :], ysb[:]
                    )

    nc.compile()
    return nc


_NC_CACHE = None


def _get_nc():
    global _NC_CACHE
    if _NC_CACHE is None:
        _NC_CACHE = build_kernel()
    return _NC_CACHE


def make_in_maps(query, query_pos, Wqc, bqc, Wqp, bqp, Wkc, bkc, Wkp, bkp, Wv, bv, Wo, bo):
    """Host-side sharding + layout prep: one input map per core."""
    query = np.asarray(query, dtype=np.float32)
    query_pos = np.asarray(query_pos, dtype=np.float32)
    shared = {
        "wqct": np.ascontiguousarray(np.asarray(Wqc, np.float32).T.astype(ml_dtypes.bfloat16)),
        "wqpt": np.ascontiguousarray(np.asarray(Wqp, np.float32).T.astype(ml_dtypes.bfloat16)),
        "wkct": np.ascontiguousarray(np.asarray(Wkc, np.float32).T.astype(ml_dtypes.bfloat16)),
        "wkpt": np.ascontiguousarray(np.asarray(Wkp, np.float32).T.astype(ml_dtypes.bfloat16)),
        "wvt": np.ascontiguousarray(np.asarray(Wv, np.float32).T.astype(ml_dtypes.bfloat16)),
        "wot": np.ascontiguousarray(np.asarray(Wo, np.float32).T.astype(ml_dtypes.bfloat16)),
        "bq": np.asarray(bqc, np.float32) + np.asarray(bqp, np.float32),
        "bk": np.asarray(bkc, np.float32) + np.asarray(bkp, np.float32),
        "bv": np.asarray(bv, np.float32),
    }
    in_maps = []
    for c in range(NCORES):
        xc = query[c * BPC : (c + 1) * BPC].reshape(T, C)
        pc = query_pos[c * BPC : (c + 1) * BPC].reshape(T, C)
        in_maps.append(
            dict(
                shared,
                xt=np.ascontiguousarray(xc.T.astype(ml_dtypes.bfloat16)),
                pt=np.ascontiguousarray(pc.T.astype(ml_dtypes.bfloat16)),
                xres=xc + np.asarray(bo, np.float32)[None, :],
            )
        )
    return in_maps


def kernel(**inputs) -> np.ndarray:
    nc = _get_nc()
    in_maps = make_in_maps(**inputs)
    res = bass_utils.run_bass_kernel_spmd(nc, in_maps, core_ids=list(range(NCORES)))
    out = np.concatenate([r["y"].reshape(BPC, L, C) for r in res.results], axis=0)
    return out


# revision 22
# speedup vs baseline: 1.0953x; 1.0087x over previous
"""Trainium2 Bass kernel for nn_ConditionalSelfAttention.

Reference computation (B=16, L=1024, C=512, H=8, D=64):
    qc = query @ Wqc.T + bqc ; qp = query_pos @ Wqp.T + bqp
    kc = query @ Wkc.T + bkc ; kp = query_pos @ Wkp.T + bkp
    v  = query @ Wv.T  + bv
    q = split_heads(qc+qp) * D**-0.5 ; k = split_heads(kc+kp)
    out = softmax(q @ k.T) @ split_heads(v)
    y = query + merge_heads(out) @ Wo.T + bo

Sharding: data-parallel over batch B across the 8 cores (2 batches/core),
no collectives.

Device dataflow (per core, per batch of 1024 tokens):
  - host pre-transposes query/query_pos to [C, T] and all weights to
    [c_in, c_out], and pre-adds bo into the residual; all matmul operands
    are fp32r (TF32-like single-pass PE mode).
  - q/k projections produce TRANSPOSED activations qT/kT [c_out, tok] by
    psum-accumulating Wc.T@X.T + Wp.T@P.T; biases are per-partition adds
    on the psum->sbuf evacuation.
  - v projection produces NATURAL layout [tok, c_out] (lhsT = X.T chunk),
    written head-major with a column of ones appended per head.
  - scores: attnT[k,q] = kT.T @ qT per head (contraction dim D=64; the two
    heads of a 128-channel pair ride different PE row-groups). exp via ACT
    with the 1/sqrt(D) scale folded in.
  - attn@V: outT[d,q] = [V|1].T @ exp_attnT accumulated over k-tiles; the
    ones column makes psum row 64 the softmax denominator. Normalization:
    reciprocal_approx_fast + DMA partition-broadcast + multiply-on-evac.
  - y = outT.T @ Wo.T + (query + bo), evacuated with the residual add.
"""

import ml_dtypes
import numpy as np

import concourse.bass as bass
import concourse.tile as tile
from concourse import bacc, mybir
from concourse import bass_utils

B, L, C, H, D = 16, 1024, 512, 8, 64
NCORES = 8
BPC = B // NCORES  # batches per core
T = BPC * L  # tokens per core
SCALE = float(D) ** -0.5
P = 128
NCT = C // P  # c-tiles (=4); also number of head pairs
NJ = L // P  # 128-token tiles per batch (=8)
f32 = mybir.dt.float32
f32r = mybir.dt.float32r
bf16 = mybir.dt.bfloat16
AL = mybir.AluOpType


def build_kernel(dbg=False):
    nc = bacc.Bacc("TRN2", debug=False, num_devices=NCORES)

    xt = nc.dram_tensor("xt", [C, T], bf16, kind="ExternalInput")
    pt = nc.dram_tensor("pt", [C, T], bf16, kind="ExternalInput")
    xres = nc.dram_tensor("xres", [T, C], f32, kind="ExternalInput")
    wqct = nc.dram_tensor("wqct", [C, C], bf16, kind="ExternalInput")
    wqpt = nc.dram_tensor("wqpt", [C, C], bf16, kind="ExternalInput")
    wkct = nc.dram_tensor("wkct", [C, C], bf16, kind="ExternalInput")
    wkpt = nc.dram_tensor("wkpt", [C, C], bf16, kind="ExternalInput")
    wvt = nc.dram_tensor("wvt", [C, C], bf16, kind="ExternalInput")
    wot = nc.dram_tensor("wot", [C, C], bf16, kind="ExternalInput")
    bq = nc.dram_tensor("bq", [C], f32, kind="ExternalInput")
    bk = nc.dram_tensor("bk", [C], f32, kind="ExternalInput")
    bv = nc.dram_tensor("bv", [C], f32, kind="ExternalInput")
    y = nc.dram_tensor("y", [T, C], f32, kind="ExternalOutput")
    if dbg:
        d_qt = nc.dram_tensor("d_qt", [P, NCT, L], bf16, kind="ExternalOutput")
        d_kt = nc.dram_tensor("d_kt", [P, NCT, L], bf16, kind="ExternalOutput")
        d_vn = nc.dram_tensor("d_vn", [P, NJ, H, D + 1], bf16, kind="ExternalOutput")
        d_exp = nc.dram_tensor("d_exp", [P, L], bf16, kind="ExternalOutput")
        d_po = nc.dram_tensor("d_po", [D + 1, 512], f32, kind="ExternalOutput")
        d_rr = nc.dram_tensor("d_rr", [1, 512], f32, kind="ExternalOutput")

    with tile.TileContext(nc) as tc:
        with (
            tc.tile_pool(name="const", bufs=1) as cpool,
            tc.tile_pool(name="xp", bufs=2) as xpool,
            tc.tile_pool(name="qk", bufs=2) as qkpool,
            tc.tile_pool(name="vn", bufs=1) as vpool,
            tc.tile_pool(name="exp", bufs=12) as epool,
            tc.tile_pool(name="osb", bufs=5) as opool,
            tc.tile_pool(name="rr", bufs=4) as rpool,
            tc.tile_pool(name="io", bufs=6) as iopool,
            tc.tile_pool(name="dsc", bufs=8, space="DRAM") as dpool,
            tc.tile_pool(name="ps", bufs=2, space="PSUM") as pspool,
            tc.tile_pool(name="pssc", bufs=2, space="PSUM") as scpool,
            tc.tile_pool(name="psout", bufs=2, space="PSUM") as povpool,
        ):
            # ---- constants ----
            def load_w(t):
                w = cpool.tile([P, NCT, C], bf16, tag=f"w_{t.name}")
                nc.sync.dma_start(w[:], t.ap().rearrange("(ko p) co -> p ko co", p=P))
                return w

            w_qc, w_qp = load_w(wqct), load_w(wqpt)
            w_kc, w_kp = load_w(wkct), load_w(wkpt)
            w_v, w_o = load_w(wvt), load_w(wot)

            bq_s = cpool.tile([P, NCT], f32, tag="bq")
            bk_s = cpool.tile([P, NCT], f32, tag="bk")
            nc.sync.dma_start(bq_s[:], bq.ap().rearrange("(ct p) -> p ct", p=P))
            nc.sync.dma_start(bk_s[:], bk.ap().rearrange("(ct p) -> p ct", p=P))
            bv_b = cpool.tile([P, C], f32, tag="bvb")
            nc.sync.dma_start(bv_b[:], bv.ap()[None, :].to_broadcast((P, C)))

            for b in range(BPC):
                tok0 = b * L
                # ---- load transposed activations for this batch ----
                xt_b = xpool.tile([P, NCT, L], bf16, tag="xt")
                pt_b = xpool.tile([P, NCT, L], bf16, tag="pt")
                nc.sync.dma_start(
                    xt_b[:],
                    xt.ap()[:, tok0 : tok0 + L].rearrange("(ko p) t -> p ko t", p=P),
                )
                nc.sync.dma_start(
                    pt_b[:],
                    pt.ap()[:, tok0 : tok0 + L].rearrange("(ko p) t -> p ko t", p=P),
                )

                # ---- q/k projections (transposed outputs) ----
                qT = qkpool.tile([P, NCT, L], bf16, tag="qT")
                kT = qkpool.tile([P, NCT, L], bf16, tag="kT")
                for ct in range(NCT):
                    for dst, wc, wp, bias in (
                        (qT, w_qc, w_qp, bq_s),
                        (kT, w_kc, w_kp, bk_s),
                    ):
                        for s in range(L // 512):
                            ps = pspool.tile([P, 512], f32, tag="ps")
                            for ko in range(NCT):
                                nc.tensor.matmul(
                                    ps[:],
                                    wc[:, ko, ct * P : (ct + 1) * P],
                                    xt_b[:, ko, s * 512 : (s + 1) * 512],
                                    start=(ko == 0),
                                    stop=False,
                                )
                            for ko in range(NCT):
                                nc.tensor.matmul(
                                    ps[:],
                                    wp[:, ko, ct * P : (ct + 1) * P],
                                    pt_b[:, ko, s * 512 : (s + 1) * 512],
                                    start=False,
                                    stop=(ko == NCT - 1),
                                )
                            nc.vector.tensor_scalar_add(
                                dst[:, ct, s * 512 : (s + 1) * 512],
                                ps[:],
                                bias[:, ct : ct + 1],
                            )

                # ---- v projection (natural layout, head-major, +ones col) ----
                v_nat = vpool.tile([P, NJ, H, D + 1], bf16, tag="vn")
                # ones column: in0*0 + 1 (memset on this strided region is
                # rejected by codegen)
                nc.vector.tensor_scalar(
                    v_nat[:, :, :, D : D + 1],
                    bv_b[:, 0 : NJ * H].rearrange("p (a b) -> p a b", b=H)[:, :, :, None],
                    0.0,
                    1.0,
                    AL.mult,
                    AL.add,
                )
                for tt in range(NJ):
                    ps = pspool.tile([P, 512], f32, tag="ps")
                    for ko in range(NCT):
                        nc.tensor.matmul(
                            ps[:],
                            xt_b[:, ko, tt * P : (tt + 1) * P],
                            w_v[:, ko, :],
                            start=(ko == 0),
                            stop=(ko == NCT - 1),
                        )
                    nc.vector.tensor_tensor(
                        v_nat[:, tt, :, 0:D],
                        ps[:].rearrange("p (h d) -> p h d", d=D),
                        bv_b[:].rearrange("p (h d) -> p h d", d=D),
                        AL.add,
                    )

                if dbg and b == 0:
                    nc.sync.dma_start(d_qt.ap(), qT[:])
                    nc.sync.dma_start(d_kt.ap(), kT[:])
                    nc.sync.dma_start(d_vn.ap(), v_nat[:])

                # ---- attention per head-pair, heads sequential ----
                out_sb = {}
                for hp in range(NCT):
                    osb = opool.tile([P, L], bf16, tag="osb")
                    out_sb[hp] = osb
                    for h01 in range(2):
                        h = hp * 2 + h01
                        prow = slice(h01 * D, (h01 + 1) * D)
                        # scores + exp per k-tile
                        exps = []
                        for j in range(NJ):
                            psc = scpool.tile([P, L], f32, tag="sc")
                            for s in range(L // 512):
                                nc.tensor.matmul(
                                    psc[:, s * 512 : (s + 1) * 512],
                                    kT[prow, hp, j * P : (j + 1) * P],
                                    qT[prow, hp, s * 512 : (s + 1) * 512],
                                    start=True,
                                    stop=True,
                                )
                            et = epool.tile([P, L], bf16, tag="exp")
                            nc.scalar.activation(
                                et[:],
                                psc[:],
                                mybir.ActivationFunctionType.Exp,
                                scale=SCALE,
                            )
                            exps.append(et)
                            if dbg and b == 0 and hp == 0 and h01 == 0 and j == 0:
                                nc.sync.dma_start(d_exp.ap(), et[:])

                        # attn @ [V|1]: accumulate over k-tiles; per-j order
                        # frees each exp slot after its two matmuls
                        pos = []
                        for s in range(L // 512):
                            po_s = povpool.tile([D + 1, 512], f32, tag="po", name=f"po_{s}")
                            pos.append(po_s)
                        for j in range(NJ):
                            for s in range(L // 512):
                                nc.tensor.matmul(
                                    pos[s][:],
                                    v_nat[:, j, h, :],
                                    exps[j][:, s * 512 : (s + 1) * 512],
                                    start=(j == 0),
                                    stop=(j == NJ - 1),
                                )
                        for s in range(L // 512):
                            po = pos[s]
                            if dbg and b == 0 and hp == 0 and h01 == 0 and s == 0:
                                dbg_sb = rpool.tile([D + 1, 512], f32, tag="rb")
                                nc.vector.tensor_copy(dbg_sb[:], po[:])
                                nc.sync.dma_start(d_po.ap(), dbg_sb[:])
                            # custom DVE ops only work at partition base 0:
                            # shift-copy the rowsum row down first
                            rraw = rpool.tile([1, 512], f32, tag="rraw")
                            nc.vector.tensor_copy(rraw[0:1, :], po[D : D + 1, :])
                            rrec = rpool.tile([1, 512], f32, tag="rrec")
                            nc.vector.reciprocal_approx_fast(rrec[:], rraw[:])
                            if dbg and b == 0 and hp == 0 and h01 == 0 and s == 0:
                                nc.sync.dma_start(d_rr.ap(), rrec[:])
                            dscr = dpool.tile([1, 512], f32, tag="dscr")
                            nc.sync.dma_start(dscr[:], rrec[:])
                            rb = rpool.tile([D, 512], f32, tag="rb")
                            nc.sync.dma_start(
                                rb[:], dscr[0:1, :].to_broadcast((D, 512))
                            )
                            nc.vector.tensor_tensor(
                                osb[prow, s * 512 : (s + 1) * 512],
                                po[0:D, :],
                                rb[:],
                                AL.mult,
                            )

                # ---- output projection + residual ----
                for tt in range(NJ):
                    psy = pspool.tile([P, 512], f32, tag="ps")
                    for hp in range(NCT):
                        nc.tensor.matmul(
                            psy[:],
                            out_sb[hp][:, tt * P : (tt + 1) * P],
                            w_o[:, hp, :],
                            start=(hp == 0),
                            stop=(hp == NCT - 1),
                        )
                    xr = iopool.tile([P, C], f32, tag="xr")
                    nc.sync.dma_start(
                        xr[:], xres.ap()[tok0 + tt * P : tok0 + (tt + 1) * P, :]
                    )
                    ysb = iopool.tile([P, C], f32, tag="ysb")
                    nc.vector.tensor_tensor(ysb[:], psy[:], xr[:], AL.add)
                    nc.sync.dma_start(
                        y.ap()[tok0 + tt * P : tok0 + (tt + 1) * P, :], ysb[:]
                    )

    nc.compile()
    return nc


_NC_CACHE = None


def _get_nc():
    global _NC_CACHE
    if _NC_CACHE is None:
        _NC_CACHE = build_kernel()
    return _NC_CACHE


def make_in_maps(query, query_pos, Wqc, bqc, Wqp, bqp, Wkc, bkc, Wkp, bkp, Wv, bv, Wo, bo):
    """Host-side sharding + layout prep: one input map per core."""
    query = np.asarray(query, dtype=np.float32)
    query_pos = np.asarray(query_pos, dtype=np.float32)
    shared = {
        "wqct": np.ascontiguousarray(np.asarray(Wqc, np.float32).T.astype(ml_dtypes.bfloat16)),
        "wqpt": np.ascontiguousarray(np.asarray(Wqp, np.float32).T.astype(ml_dtypes.bfloat16)),
        "wkct": np.ascontiguousarray(np.asarray(Wkc, np.float32).T.astype(ml_dtypes.bfloat16)),
        "wkpt": np.ascontiguousarray(np.asarray(Wkp, np.float32).T.astype(ml_dtypes.bfloat16)),
        "wvt": np.ascontiguousarray(np.asarray(Wv, np.float32).T.astype(ml_dtypes.bfloat16)),
        "wot": np.ascontiguousarray(np.asarray(Wo, np.float32).T.astype(ml_dtypes.bfloat16)),
        "bq": np.asarray(bqc, np.float32) + np.asarray(bqp, np.float32),
        "bk": np.asarray(bkc, np.float32) + np.asarray(bkp, np.float32),
        "bv": np.asarray(bv, np.float32),
    }
    in_maps = []
    for c in range(NCORES):
        xc = query[c * BPC : (c + 1) * BPC].reshape(T, C)
        pc = query_pos[c * BPC : (c + 1) * BPC].reshape(T, C)
        in_maps.append(
            dict(
                shared,
                xt=np.ascontiguousarray(xc.T.astype(ml_dtypes.bfloat16)),
                pt=np.ascontiguousarray(pc.T.astype(ml_dtypes.bfloat16)),
                xres=xc + np.asarray(bo, np.float32)[None, :],
            )
        )
    return in_maps


def kernel(**inputs) -> np.ndarray:
    nc = _get_nc()
    in_maps = make_in_maps(**inputs)
    res = bass_utils.run_bass_kernel_spmd(nc, in_maps, core_ids=list(range(NCORES)))
    out = np.concatenate([r["y"].reshape(BPC, L, C) for r in res.results], axis=0)
    return out
